# revision 44
# baseline (speedup 1.0000x reference)
"""Bahdanau attention kernel for Trainium2 (Bass/Tile), SPMD over 8 NeuronCores.

Reference computation (per example b):
    q_proj = query @ W1_k + W1_b                     # [U]
    v_proj = values @ W2_k + W2_b                    # [T, U]
    score  = tanh(q_proj + v_proj) @ V_k + V_b       # [T]
    attn   = softmax(score)                          # over T
    out    = sum_t attn[t] * values[t]               # [D]

Sharding: data-parallel over batch B=32 -> 4 examples per core; params
replicated. No collectives.

Per-core dataflow:
  - The dominant matmul (v_proj: [T,D]x[D,U], 99.8% of FLOPs) runs in
    fp8-e4m3 with MatmulPerfMode.DoubleRow: both operands are quantized
    host-side with power-of-2 absmax scales, and each PE pass contracts two
    128-row k-tiles at once (2x bf16 throughput, ~109us/core at roofline).
    The 1/(sv*sw) descale is folded into the tanh activation's scale operand.
  - values arrive pre-transposed from the host in [p, kt, t] layout, twice:
    fp8 (matmul operand) and bf16 (exact copy for the context step), so every
    DMA is a plain contiguous load (16-32KB per partition, no xbar transpose).
  - qb = query @ W1 + W1_b + W2_b is folded host-side (fp32) into a per-
    (u-partition, example) bias tensor -- standard bias folding, O(B*D*U)
    which is 0.2% of the FLOPs.
  - each u-tile accumulates its 4 T-chunks into ONE [P, 2048] psum tile
    (4 banks); a single merged tanh activation consumes it (halves ScalarE
    instruction overhead vs per-chunk activations).
  - score[t] = sum_u V_u*tanh[u,t] splits across engines: u-tiles 0..3 chain
    on DVE (scalar_tensor_tensor with per-partition V), u-tiles 4..7 are
    V-scaled into g tiles at 2x tensor_scalar rate and summed by the PE's
    score matmul (5 accumulating ones-matmuls per 512-chunk, which also
    broadcast the score to all 128 partitions for the softmax). This keeps
    DVE's pace below the PE's so psum banks recycle on time.
  - the score matmuls write into ut7's own psum banks (write-after-tanh) and
    the whole score/softmax/context phase of example b is emitted after
    example b+1's first u-tile: the bank<->tile mapping stays periodic and
    the scheduler never splits the weight-stationary j-groups (which would
    cost a ~140ns LDWEIGHTS per matmul).
  - softmax without max subtraction (|score| <= sum|V_k| ~ 16, safe in fp32);
    ONE merged exp activation with fused accumulated sum-of-exp.
  - context[d] = sum_t attn[t]*valuesT_bf16[d,t] in one DVE pass per d-tile
    via scalar_tensor_tensor(mult, mult, accum_out) over [128, 2048] bf16.
  - TAIL: the last example's context runs as a PE matvec over natural-layout
    values: its score is computed at M=1 (no broadcast needed), transposed
    into [t-partition] layout by tiny K=1 matmuls, exponentiated on ScalarE,
    and contracted against the values with N=512 matmuls. The second-to-last
    example's DVE context is deferred behind the last example's chain ops --
    so the final ~30us runs PE and DVE concurrently instead of serially on
    DVE. Score matmuls are emitted summand-outer so the 16 early-ready ones
    fill the PE while the last tanh/g tile is still in flight.
  - startup: w28/v8[0] DMAs are interleaved in k-pair chunks so the first
    j-group starts after ~3us of data instead of ~10us.
"""

import sys

_REPO = "/opt/trn_rl_repo"
if _REPO not in sys.path:
    sys.path.insert(0, _REPO)

import numpy as np
from contextlib import ExitStack

import concourse.bass as bass
import concourse.tile as tile
from concourse import mybir
from concourse import bass_utils as _bass_utils
from concourse.bass_utils import run_bass_kernel_spmd
from concourse.masks import make_identity

# walrus defaults to --enable-ldw-opt=false; true lets codegen elide/pipeline
# LDWEIGHTS so the scheduler's bank-chasing matmul order stops paying ~140ns
# per reload.
_orig_run_command = _bass_utils.run_command


def _patched_run_command(cmd, **kw):
    # --enable-ldw-opt=true crashes walrus visitInstLdweights with explicit
    # (DoubleRow) ldweights in the stream; keep the default.
    return _orig_run_command(cmd, **kw)


_bass_utils.run_command = _patched_run_command

B, T, D, U = 32, 2048, 1024, 1024
NCORES = 8
BL = B // NCORES  # 4 examples per core

P = 128
KT = D // P       # 8 contraction tiles over d
UT = U // P       # 8 tiles over u
PAIRS = KT // 2   # 4 DoubleRow k-tile pairs
NQ = 4            # T chunks for the N=512 matmuls
QT = T // NQ      # 512
NCHAIN = 4        # u-tiles whose V-reduce chains on DVE; the rest sum on PE

F32 = mybir.dt.float32
BF16 = mybir.dt.bfloat16
F8 = mybir.dt.float8e4
ADD = mybir.AluOpType.add
MULT = mybir.AluOpType.mult
AF = mybir.ActivationFunctionType
AX = mybir.AxisListType
DR = mybir.MatmulPerfMode.DoubleRow


def _emit(ctx: ExitStack, tc: tile.TileContext, qb, v8, vbf, vnat, w28, vk, desc, out):
    nc = tc.nc

    singles = ctx.enter_context(tc.tile_pool(name="singles", bufs=1))
    v8p = ctx.enter_context(tc.tile_pool(name="v8p", bufs=2))
    vbfp = ctx.enter_context(tc.tile_pool(name="vbfp", bufs=2))
    thp = ctx.enter_context(tc.tile_pool(name="thp", bufs=5))
    accp = ctx.enter_context(tc.tile_pool(name="accp", bufs=3))
    gp = ctx.enter_context(tc.tile_pool(name="gp", bufs=5))
    attnp = ctx.enter_context(tc.tile_pool(name="attnp", bufs=2))
    scrp = ctx.enter_context(tc.tile_pool(name="scrp", bufs=2))
    smallp = ctx.enter_context(tc.tile_pool(name="smallp", bufs=4))
    # 2 bufs x [P, 2048] f32 = 4 PSUM banks each = all 8 banks
    vpsum = ctx.enter_context(tc.tile_pool(name="vpsum", bufs=2, space="PSUM"))

    # ---- replicated parameters + first-example tiles -----------------------
    # The startup DMAs serialize on one queue, and ut0's j-loop consumes
    # w28/v8[0] one k-pair at a time -- so interleave pair-sized chunks of
    # both (w28-pair j, v8-pair j, ...) and the PE starts after the first
    # pair (~3us of data) instead of after both full tiles (~10us).
    w28_sb = singles.tile([P, KT, U], F8, tag="w28", name="w28_sb")

    v8_t, vbf_t = {}, {}

    def load_v8(b):
        t = v8p.tile([P, KT, T], F8, tag="v8", name=f"v8_{b}")
        nc.sync.dma_start(out=t[:], in_=v8[b])
        v8_t[b] = t

    def load_vbf(b):
        t = vbfp.tile([P, KT, T], BF16, tag="vbf", name=f"vbf_{b}")
        nc.sync.dma_start(out=t[:], in_=vbf[b])
        vbf_t[b] = t

    # natural-layout values for the LAST example (t on partitions): its
    # context runs as a PE matvec instead of on the saturated DVE tail.
    # Same byte size as a vbf tile, so it borrows the vbf pool slot.
    vnat_t = {}

    def load_vnat():
        t = vbfp.tile([P, T // P, D], BF16, tag="vbf", name="vnat")
        nc.sync.dma_start(out=t[:], in_=vnat)
        vnat_t[0] = t

    v8_0 = v8p.tile([P, KT, T], F8, tag="v8", name="v8_0")
    v8_t[0] = v8_0
    for j in range(PAIRS):
        nc.sync.dma_start(
            out=w28_sb[:, 2 * j : 2 * j + 2, :], in_=w28[:, 2 * j : 2 * j + 2, :]
        )
        nc.sync.dma_start(
            out=v8_0[:, 2 * j : 2 * j + 2, :], in_=v8[0][:, 2 * j : 2 * j + 2, :]
        )

    qb_sb = singles.tile([P, UT, BL], F32, tag="qb", name="qb_sb")
    nc.sync.dma_start(out=qb_sb[:], in_=qb)
    vk_sb = singles.tile([P, UT], F32, tag="vk", name="vk_sb")
    nc.sync.dma_start(out=vk_sb[:], in_=vk)
    desc_sb = singles.tile([P, 1], F32, tag="desc", name="desc_sb")
    nc.sync.dma_start(out=desc_sb[:], in_=desc)

    load_vbf(0)

    ones_sb = singles.tile([P, P], BF16, tag="ones", name="ones_sb")
    nc.vector.memset(ones_sb[:], 1.0)
    ident32 = singles.tile([P, P], F32, tag="ident32", name="ident32")
    make_identity(nc, ident32[:])

    # context accumulator for all local examples, [d_inner, b, d_tile]
    ctx_all = singles.tile([P, BL, KT], F32, tag="ctxall", name="ctx_all")

    # ---- main per-example pipeline ----------------------------------------
    # The score/softmax/context phase for example b is EMITTED after example
    # b+1's first u-tile: its 20+4 score matmuls then sit after ut0's in the
    # scheduler's priority order and execute as one clean bundle at the
    # ut0/ut1 boundary (Exp drains while the PE runs the score matmuls), so
    # the weight-stationary j-groups of the big matmul never get split.
    pending = {}

    pending2 = {}

    def emit_context(b):
        attn, rcp, vbf_tile = pending2.pop(b)
        # context[d] = (1/sumexp) * sum_t exp(score_t) * valuesT[d, t]
        ctxp_t = smallp.tile([P, KT], F32, tag="ctxp", name="ctxp_t")
        for dt in range(KT):
            scr = scrp.tile([P, T], BF16, tag="scr", name="scr")
            nc.vector.scalar_tensor_tensor(
                out=scr[:],
                in0=vbf_tile[:, dt, :],
                scalar=1.0,
                in1=attn[:],
                op0=MULT,
                op1=MULT,
                accum_out=ctxp_t[:, dt : dt + 1],
            )
        nc.vector.tensor_scalar_mul(out=ctx_all[:, b, :], in0=ctxp_t[:], scalar1=rcp[:])

    def emit_score_softmax_context(b, score_ps, defer_context=False):
        acc, gs, vbf_tile = pending.pop(b)
        # score[t] = sum_p (accA + g4 + ... + g7)[p, t], broadcast to 128
        # partitions, then softmax over T (no max subtraction;
        # |score| <= sum|V_k| ~ 16). The broadcast reuses ut7's psum banks
        # (write-after-tanh) so the score phase allocates NO extra psum
        # tiles and the bank<->tile mapping stays periodic across examples.
        summands = [acc] + gs
        attn = attnp.tile([P, T], BF16, tag="attn", name="attn")
        sumexp = smallp.tile([P, 1], F32, tag="sumexp", name="sumexp")
        for si, s in enumerate(summands):
            for c in range(NQ):
                nc.tensor.matmul(
                    score_ps[:, c * QT : (c + 1) * QT],
                    lhsT=ones_sb[:],
                    rhs=s[:, c * QT : (c + 1) * QT],
                    start=(si == 0),
                    stop=(si == len(summands) - 1),
                )
        nc.scalar.activation(
            out=attn[:],
            in_=score_ps[:],
            func=AF.Exp,
            accum_out=sumexp[:],
        )
        rcp = smallp.tile([P, 1], F32, tag="rcp", name="rcp")
        nc.vector.reciprocal(out=rcp[:], in_=sumexp[:])

        pending2[b] = (attn, rcp, vbf_tile)
        if not defer_context:
            emit_context(b)

    for b in range(BL):
        prefetch = []
        if b + 1 < BL:
            ld2 = load_vbf if b + 1 < BL - 1 else (lambda _b: load_vnat())
            prefetch = [(load_v8, b + 1), (ld2, b + 1)]

        # score partials: V-weighted tanh tiles. ut 0..3 are chained on DVE
        # into accA; ut 4..7 stay as separate g tiles (pre-multiplied by V at
        # 2x tensor_scalar rate) and the PE's score matmul sums them -- this
        # keeps the post-tanh serial DVE latency off the critical path.
        acc = None
        gs = []
        psv0 = None
        for ut in range(UT):
            if ut in (2, 5) and prefetch:
                fn, arg = prefetch.pop(0)
                fn(arg)
            # v_projT[u, t] for this u-tile: fp8 DoubleRow, 2 k-tiles/pass.
            # One [P, 2048] psum tile spans 4 banks; the c-chunks are
            # independent 512-wide accumulation groups within it, consumed by
            # a single merged tanh activation.
            psv = vpsum.tile([P, T], F32, tag="vp", name="psv")
            for j in range(PAIRS):
                for c in range(NQ):
                    nc.tensor.matmul(
                        psv[:, c * QT : (c + 1) * QT],
                        lhsT=w28_sb[:, 2 * j : 2 * j + 2, ut * P : (ut + 1) * P],
                        rhs=v8_t[b][:, 2 * j : 2 * j + 2, c * QT : (c + 1) * QT],
                        start=(j == 0),
                        stop=(j == PAIRS - 1),
                        perf_mode=DR,
                    )
            th = thp.tile([P, T], BF16, tag="th", name="th")
            nc.scalar.activation(
                out=th[:],
                in_=psv[:],
                func=AF.Tanh,
                bias=qb_sb[:, ut, b : b + 1],
                scale=desc_sb[:, 0:1],
            )
            if ut == 0:
                psv0 = psv
            if ut == 0:
                acc = accp.tile([P, T], BF16, tag="acc", name="acc")
                nc.vector.tensor_scalar(
                    out=acc[:], in0=th[:], scalar1=vk_sb[:, 0:1], scalar2=None,
                    op0=MULT,
                )
            elif ut < NCHAIN:
                nacc = accp.tile([P, T], BF16, tag="acc", name="acc")
                nc.vector.scalar_tensor_tensor(
                    out=nacc[:],
                    in0=th[:],
                    scalar=vk_sb[:, ut : ut + 1],
                    in1=acc[:],
                    op0=MULT,
                    op1=ADD,
                )
                acc = nacc
            else:
                g = gp.tile([P, T], BF16, tag="g", name=f"g{ut}")
                nc.vector.tensor_scalar(
                    out=g[:], in0=th[:], scalar1=vk_sb[:, ut : ut + 1], scalar2=None,
                    op0=MULT,
                )
                gs.append(g)
            if ut == 1 and b > 0:
                emit_score_softmax_context(
                    b - 1, psv0, defer_context=(b == BL - 1)
                )
        while prefetch:
            fn, arg = prefetch.pop(0)
            fn(arg)
        pending[b] = (acc, gs, vbf_t.get(b))
    emit_context(BL - 2)

    # ---- last example: score -> bf16 copy -> scoreT via N=1 matmuls ->
    # exp(scale=1/128) -> PE matvec over natural-layout values ------------
    b = BL - 1
    acc, gs, _ = pending.pop(b)
    summands = [acc] + gs
    final_ps = vpsum.tile([P, T], F32, tag="vp", name="final_ps")
    for si, s in enumerate(summands):
        for c in range(NQ):
            nc.tensor.matmul(
                final_ps[0:1, c * QT : (c + 1) * QT],
                lhsT=ones_sb[:, 0:1],
                rhs=s[:, c * QT : (c + 1) * QT],
                start=(si == 0),
                stop=(si == len(summands) - 1),
            )
    # psum -> sbuf bf16 score copy, chunked so the scoreT matmuls can trail
    score3 = attnp.tile([P, T], BF16, tag="attn", name="score3")
    for c in range(NQ):
        nc.scalar.activation(
            out=score3[0:1, c * QT : (c + 1) * QT],
            in_=final_ps[0:1, c * QT : (c + 1) * QT],
            func=AF.Copy,
        )
    aux = vpsum.tile([P, T], F32, tag="vp", name="aux")
    TCH = T // P  # 16 t-chunks
    # all broadcast rows are identical, so a ones-column reduce gives
    # 128*score per t-lane; the exact 1/128 folds into exp's scale
    for tc in range(TCH):
        nc.tensor.matmul(
            aux[:, tc : tc + 1],
            lhsT=score3[0:1, tc * P : (tc + 1) * P],
            rhs=ones_sb[0:1, 0:1],
            start=True,
            stop=True,
        )
    attnT = smallp.tile([P, TCH], BF16, tag="attnT", name="attnT")
    nc.scalar.activation(out=attnT[:], in_=aux[:, 0:TCH], func=AF.Exp)
    # sum of exp: partition-reduce attnT, then reduce the 16 chunk sums
    nc.tensor.matmul(
        aux[0:1, QT : QT + TCH], lhsT=ones_sb[:, 0:1], rhs=attnT[:],
        start=True, stop=True,
    )
    se3 = smallp.tile([1, 1], F32, tag="se3", name="se3")
    nc.vector.reduce_sum(out=se3[:], in_=aux[0:1, QT : QT + TCH], axis=AX.X)
    rcp3 = smallp.tile([1, 1], F32, tag="rcp3", name="rcp3")
    nc.vector.reciprocal(out=rcp3[:], in_=se3[:])
    # context matvec: ctx[d] = sum_tc sum_t attnT[t, tc] * vnat[t, tc, d]
    vn = vnat_t[0]
    for tc in range(TCH):
        for h in range(2):
            nc.tensor.matmul(
                aux[0:1, (2 + h) * QT : (2 + h) * QT + QT],
                lhsT=attnT[:, tc : tc + 1],
                rhs=vn[:, tc, h * QT : (h + 1) * QT],
                start=(tc == 0),
                stop=(tc == TCH - 1),
            )
    ctx3 = smallp.tile([1, D], F32, tag="ctx3", name="ctx3")
    nc.scalar.activation(
        out=ctx3[:], in_=aux[0:1, 2 * QT : 2 * QT + D], func=AF.Copy,
        scale=rcp3[0:1, 0:1],
    )
    nc.sync.dma_start(out=out[b : b + 1], in_=ctx3[:])

    # ---- write out examples 0..BL-2: transpose -> contiguous rows --------
    nb = BL - 1
    pso = vpsum.tile([P, T], F32, tag="vp", name="pso")
    nc.tensor.transpose(
        pso[: nb * KT, :P],
        ctx_all[:, :nb, :].rearrange("p b k -> p (b k)"),
        ident32[:],
    )
    ctxT = smallp.tile([nb * KT, P], F32, tag="ctxT", name="ctxT")
    nc.vector.tensor_copy(out=ctxT[:], in_=pso[: nb * KT, :P])
    nc.sync.dma_start(
        out=out[:nb].rearrange("b (dt p) -> (b dt) p", p=P), in_=ctxT[:]
    )


def _dedupe_ldweights(nc: bass.Bass) -> int:
    """Replace an InstLdweights whose stationary operand is identical to the
    previous InstLdweights on the same engine (with only matmuls in between)
    by a NoOp carrying the same name + sync_info. The PE weight registers
    persist across matmuls, so reloading the same tile is pure overhead
    (~114 ns each) that serializes with the matmul stream.

    Semaphore waits/updates, drains, noops, and register moves do not touch
    the PE weight registers, so tracking survives them; the SBUF region the
    weights were loaded from is only safe to skip re-reading because weight
    tiles here (w28_sb, ones_sb, ident32) are written once and never reused
    for anything else."""
    TRANSPARENT = {
        "InstEventSemaphore",
        "InstDrain",
        "InstNoOp",
        "InstRegisterMove",
    }
    n = 0
    for f in nc.m.functions:
        for blk in f.blocks:
            il = blk.instructions
            last_sig = {}
            out = []
            for inst in il:
                tn = type(inst).__name__
                eng = getattr(inst, "engine", None)
                if tn == "InstLdweights":
                    op = inst.ins[0]
                    sig = (
                        getattr(op, "memref", None),
                        getattr(op, "offset", None),
                        str(getattr(op, "ap", None)),
                        str(getattr(op, "dtype", None)),
                        str(inst.is_transpose),
                        str(inst.perf_mode),
                        str(inst.tile_position),
                    )
                    if last_sig.get(eng) == sig:
                        out.append(
                            mybir.InstNoOp(
                                name=inst.name,
                                engine=inst.engine,
                                ins=[],
                                outs=[],
                                sync_info=inst.sync_info,
                            )
                        )
                        n += 1
                        continue
                    last_sig[eng] = sig
                elif (
                    tn != "InstMatmult" and tn not in TRANSPARENT and eng is not None
                ):
                    # anything else on this engine invalidates tracking
                    last_sig.pop(eng, None)
                out.append(inst)
            il[:] = out
    return n


def _split_multi_waits(nc: bass.Bass) -> int:
    """The walrus build here accepts only ONE semaphore wait per instruction;
    hoist extra waits onto single-wait NoOps preceding the instruction (same
    engine, in-order, so semantics are preserved)."""
    n_split = 0
    for f in nc.m.functions:
        for b in f.blocks:
            il = b.instructions
            out, changed = [], False
            for inst in il:
                si = inst.sync_info
                waits = list(si.on_wait) if (si and si.on_wait) else []
                if len(waits) > 1:
                    changed = True
                    n_split += 1
                    for j, w in enumerate(waits[:-1]):
                        out.append(
                            mybir.InstNoOp(
                                name=f"{inst.name}.sw{j}",
                                engine=inst.engine,
                                ins=[],
                                outs=[],
                                sync_info=mybir.SyncInfo(on_wait=[w], on_update=[]),
                            )
                        )
                    inst.sync_info = mybir.SyncInfo(
                        on_wait=[waits[-1]], on_update=list(si.on_update or [])
                    )
                out.append(inst)
            if changed:
                il[:] = out
    return n_split


def build_program(split_waits: bool = True) -> bass.Bass:
    nc = bass.Bass("TRN2", target_bir_lowering=False, debug=False, num_devices=NCORES)
    qb_h = nc.dram_tensor("qb", [P, UT, BL], F32, kind="ExternalInput")
    v8_h = nc.dram_tensor("values8", [BL, P, KT, T], F8, kind="ExternalInput")
    vbf_h = nc.dram_tensor("valuesbf", [BL, P, KT, T], BF16, kind="ExternalInput")
    vnat_h = nc.dram_tensor("valuesnat", [P, T // P, D], BF16, kind="ExternalInput")
    w28_h = nc.dram_tensor("W2_8", [P, KT, U], F8, kind="ExternalInput")
    vk_h = nc.dram_tensor("Vk", [P, UT], F32, kind="ExternalInput")
    desc_h = nc.dram_tensor("descale", [P, 1], F32, kind="ExternalInput")
    out_h = nc.dram_tensor("context", [BL, D], F32, kind="ExternalOutput")
    with tile.TileContext(nc) as tc:
        with ExitStack() as ctx:
            _emit(
                ctx, tc,
                qb_h.ap(), v8_h.ap(), vbf_h.ap(), vnat_h.ap(),
                w28_h.ap(), vk_h.ap(), desc_h.ap(),
                out_h.ap(),
            )
    if split_waits:
        _dedupe_ldweights(nc)
        _split_multi_waits(nc)
    return nc


_PROGRAM = None


def _get_program() -> bass.Bass:
    global _PROGRAM
    if _PROGRAM is None:
        _PROGRAM = build_program()
    return _PROGRAM


def _pow2_scale(absmax: float, target: float = 224.0) -> float:
    return float(2.0 ** np.floor(np.log2(target / absmax)))


def make_in_maps(inputs: dict) -> list[dict]:
    import ml_dtypes

    F8NP = ml_dtypes.float8_e4m3
    BFNP = ml_dtypes.bfloat16

    query = np.asarray(inputs["query"], dtype=np.float32)
    values = np.asarray(inputs["values"], dtype=np.float32)
    W1 = np.asarray(inputs["W1_k"], dtype=np.float32)
    W1b = np.asarray(inputs["W1_b"], dtype=np.float32)
    W2 = np.asarray(inputs["W2_k"], dtype=np.float32)
    W2b = np.asarray(inputs["W2_b"], dtype=np.float32)
    Vk = np.asarray(inputs["V_k"], dtype=np.float32)

    sv = _pow2_scale(float(np.abs(values).max()))
    sw = _pow2_scale(float(np.abs(W2).max()))

    # values, transposed to [b, p, kt, t]
    vT = np.ascontiguousarray(values.transpose(0, 2, 1))  # [B, D, T]
    vT = vT.reshape(B, KT, P, T).swapaxes(1, 2)  # [B, P, KT, T] view
    vbf = vT.astype(BFNP)
    v8 = (vT * np.float32(sv)).astype(F8NP)

    # W2 * sw, [p, kt, u] fp8
    w28 = (W2 * np.float32(sw)).reshape(KT, P, U).swapaxes(0, 1).astype(F8NP)
    w28 = np.ascontiguousarray(w28)

    # qb = query @ W1 + W1_b + W2_b, folded host-side (fp32), [p, ut, b]
    qb_full = query @ W1 + (W1b + W2b)[None, :]  # [B, U]

    vk = np.ascontiguousarray(Vk[:, 0].reshape(UT, P).T, dtype=np.float32)
    desc = np.full((P, 1), 1.0 / (sv * sw), dtype=np.float32)

    TCH = T // P
    in_maps = []
    for c in range(NCORES):
        sl = slice(c * BL, (c + 1) * BL)
        vnat_c = np.ascontiguousarray(
            values[c * BL + BL - 1].reshape(TCH, P, D).swapaxes(0, 1).astype(BFNP)
        )
        qb_c = np.ascontiguousarray(
            qb_full[sl].T.reshape(UT, P, BL).swapaxes(0, 1), dtype=np.float32
        )
        in_maps.append(
            {
                "qb": qb_c,
                "valuesnat": vnat_c,
                "values8": np.ascontiguousarray(v8[sl]),
                "valuesbf": np.ascontiguousarray(vbf[sl]),
                "W2_8": w28,
                "Vk": vk,
                "descale": desc,
            }
        )
    return in_maps


def kernel(**inputs) -> np.ndarray:
    nc = _get_program()
    res = run_bass_kernel_spmd(nc, make_in_maps(inputs), list(range(NCORES))).results
    return np.concatenate([res[c]["context"] for c in range(NCORES)], axis=0)


if __name__ == "__main__":
    rng = np.random.default_rng(0)
    inputs = {
        "query": rng.standard_normal((B, D), dtype=np.float32),
        "values": rng.standard_normal((B, T, D), dtype=np.float32),
        "W1_k": (rng.standard_normal((D, U)) * 0.02).astype(np.float32),
        "W1_b": np.zeros(U, np.float32),
        "W2_k": (rng.standard_normal((D, U)) * 0.02).astype(np.float32),
        "W2_b": np.zeros(U, np.float32),
        "V_k": (rng.standard_normal((U, 1)) * 0.02).astype(np.float32),
        "V_b": np.zeros(1, np.float32),
    }
    out = kernel(**inputs)
    print(out.shape, out.dtype)


# revision 45
# speedup vs baseline: 1.0139x; 1.0139x over previous
"""Bahdanau attention kernel for Trainium2 (Bass/Tile), SPMD over 8 NeuronCores.

Reference computation (per example b):
    q_proj = query @ W1_k + W1_b                     # [U]
    v_proj = values @ W2_k + W2_b                    # [T, U]
    score  = tanh(q_proj + v_proj) @ V_k + V_b       # [T]
    attn   = softmax(score)                          # over T
    out    = sum_t attn[t] * values[t]               # [D]

Sharding: data-parallel over batch B=32 -> 4 examples per core; params
replicated. No collectives.

Per-core dataflow:
  - The dominant matmul (v_proj: [T,D]x[D,U], 99.8% of FLOPs) runs in
    fp8-e4m3 with MatmulPerfMode.DoubleRow: both operands are quantized
    host-side with power-of-2 absmax scales, and each PE pass contracts two
    128-row k-tiles at once (2x bf16 throughput, ~109us/core at roofline).
    The 1/(sv*sw) descale is folded into the tanh activation's scale operand.
  - values arrive pre-transposed from the host in [p, kt, t] layout, twice:
    fp8 (matmul operand) and bf16 (exact copy for the context step), so every
    DMA is a plain contiguous load (16-32KB per partition, no xbar transpose).
  - qb = query @ W1 + W1_b + W2_b is folded host-side (fp32) into a per-
    (u-partition, example) bias tensor -- standard bias folding, O(B*D*U)
    which is 0.2% of the FLOPs.
  - each u-tile accumulates its 4 T-chunks into ONE [P, 2048] psum tile
    (4 banks); a single merged tanh activation consumes it (halves ScalarE
    instruction overhead vs per-chunk activations).
  - score[t] = sum_u V_u*tanh[u,t] splits across engines: u-tiles 0..3 chain
    on DVE (scalar_tensor_tensor with per-partition V), u-tiles 4..7 are
    V-scaled into g tiles at 2x tensor_scalar rate and summed by the PE's
    score matmul (5 accumulating ones-matmuls per 512-chunk, which also
    broadcast the score to all 128 partitions for the softmax). This keeps
    DVE's pace below the PE's so psum banks recycle on time.
  - the score matmuls write into ut7's own psum banks (write-after-tanh) and
    the whole score/softmax/context phase of example b is emitted after
    example b+1's first u-tile: the bank<->tile mapping stays periodic and
    the scheduler never splits the weight-stationary j-groups (which would
    cost a ~140ns LDWEIGHTS per matmul).
  - softmax without max subtraction (|score| <= sum|V_k| ~ 16, safe in fp32);
    ONE merged exp activation with fused accumulated sum-of-exp.
  - context[d] = sum_t attn[t]*valuesT_bf16[d,t] in one DVE pass per d-tile
    via scalar_tensor_tensor(mult, mult, accum_out) over [128, 2048] bf16.
  - TAIL: the last example's context runs as a PE matvec over natural-layout
    values: its score is computed at M=1 (no broadcast needed), transposed
    into [t-partition] layout by tiny K=1 matmuls, exponentiated on ScalarE,
    and contracted against the values with N=512 matmuls. The second-to-last
    example's DVE context is deferred behind the last example's chain ops --
    so the final ~30us runs PE and DVE concurrently instead of serially on
    DVE. Score matmuls are emitted summand-outer so the 16 early-ready ones
    fill the PE while the last tanh/g tile is still in flight.
  - startup: w28/v8[0] DMAs are interleaved in k-pair chunks so the first
    j-group starts after ~3us of data instead of ~10us.
"""

import sys

_REPO = "/opt/trn_rl_repo"
if _REPO not in sys.path:
    sys.path.insert(0, _REPO)

import numpy as np
from contextlib import ExitStack

import concourse.bass as bass
import concourse.tile as tile
from concourse import mybir
from concourse import bass_utils as _bass_utils
from concourse.bass_utils import run_bass_kernel_spmd
from concourse.masks import make_identity

# walrus defaults to --enable-ldw-opt=false; true lets codegen elide/pipeline
# LDWEIGHTS so the scheduler's bank-chasing matmul order stops paying ~140ns
# per reload.
_orig_run_command = _bass_utils.run_command


def _patched_run_command(cmd, **kw):
    # --enable-ldw-opt=true crashes walrus visitInstLdweights with explicit
    # (DoubleRow) ldweights in the stream; keep the default.
    return _orig_run_command(cmd, **kw)


_bass_utils.run_command = _patched_run_command

B, T, D, U = 32, 2048, 1024, 1024
NCORES = 8
BL = B // NCORES  # 4 examples per core

P = 128
KT = D // P       # 8 contraction tiles over d
UT = U // P       # 8 tiles over u
PAIRS = KT // 2   # 4 DoubleRow k-tile pairs
NQ = 4            # T chunks for the N=512 matmuls
QT = T // NQ      # 512
NCHAIN = 4        # u-tiles whose V-reduce chains on DVE; the rest sum on PE

F32 = mybir.dt.float32
BF16 = mybir.dt.bfloat16
F8 = mybir.dt.float8e4
ADD = mybir.AluOpType.add
MULT = mybir.AluOpType.mult
AF = mybir.ActivationFunctionType
AX = mybir.AxisListType
DR = mybir.MatmulPerfMode.DoubleRow


def _emit(ctx: ExitStack, tc: tile.TileContext, qb, v8, vbf, vnat, w28, vk, desc, out):
    nc = tc.nc

    singles = ctx.enter_context(tc.tile_pool(name="singles", bufs=1))
    v8p = ctx.enter_context(tc.tile_pool(name="v8p", bufs=2))
    vbfp = ctx.enter_context(tc.tile_pool(name="vbfp", bufs=2))
    thp = ctx.enter_context(tc.tile_pool(name="thp", bufs=5))
    accp = ctx.enter_context(tc.tile_pool(name="accp", bufs=3))
    gp = ctx.enter_context(tc.tile_pool(name="gp", bufs=5))
    attnp = ctx.enter_context(tc.tile_pool(name="attnp", bufs=2))
    scrp = ctx.enter_context(tc.tile_pool(name="scrp", bufs=2))
    smallp = ctx.enter_context(tc.tile_pool(name="smallp", bufs=4))
    # 2 bufs x [P, 2048] f32 = 4 PSUM banks each = all 8 banks
    vpsum = ctx.enter_context(tc.tile_pool(name="vpsum", bufs=2, space="PSUM"))

    # ---- replicated parameters + first-example tiles -----------------------
    # The startup DMAs serialize on one queue, and ut0's j-loop consumes
    # w28/v8[0] one k-pair at a time -- so interleave pair-sized chunks of
    # both (w28-pair j, v8-pair j, ...) and the PE starts after the first
    # pair (~3us of data) instead of after both full tiles (~10us).
    w28_sb = singles.tile([P, KT, U], F8, tag="w28", name="w28_sb")

    v8_t, vbf_t = {}, {}

    def load_v8(b):
        t = v8p.tile([P, KT, T], F8, tag="v8", name=f"v8_{b}")
        nc.sync.dma_start(out=t[:], in_=v8[b])
        v8_t[b] = t

    def load_vbf(b):
        t = vbfp.tile([P, KT, T], BF16, tag="vbf", name=f"vbf_{b}")
        nc.sync.dma_start(out=t[:], in_=vbf[b])
        vbf_t[b] = t

    # natural-layout values for the LAST example (t on partitions): its
    # context runs as a PE matvec instead of on the saturated DVE tail.
    # Same byte size as a vbf tile, so it borrows the vbf pool slot.
    vnat_t = {}

    def load_vnat():
        t = vbfp.tile([P, T // P, D], BF16, tag="vbf", name="vnat")
        nc.sync.dma_start(out=t[:], in_=vnat)
        vnat_t[0] = t

    v8_0 = v8p.tile([P, KT, T], F8, tag="v8", name="v8_0")
    v8_t[0] = v8_0
    for j in range(PAIRS):
        nc.sync.dma_start(
            out=w28_sb[:, 2 * j : 2 * j + 2, :], in_=w28[:, 2 * j : 2 * j + 2, :]
        )
        nc.sync.dma_start(
            out=v8_0[:, 2 * j : 2 * j + 2, :], in_=v8[0][:, 2 * j : 2 * j + 2, :]
        )

    qb_sb = singles.tile([P, UT, BL], F32, tag="qb", name="qb_sb")
    nc.sync.dma_start(out=qb_sb[:], in_=qb)
    vk_sb = singles.tile([P, UT], F32, tag="vk", name="vk_sb")
    nc.sync.dma_start(out=vk_sb[:], in_=vk)
    desc_sb = singles.tile([P, 1], F32, tag="desc", name="desc_sb")
    nc.sync.dma_start(out=desc_sb[:], in_=desc)

    load_vbf(0)

    ones_sb = singles.tile([P, P], BF16, tag="ones", name="ones_sb")
    nc.vector.memset(ones_sb[:], 1.0)
    ident32 = singles.tile([P, P], F32, tag="ident32", name="ident32")
    make_identity(nc, ident32[:])

    # context accumulator for all local examples, [d_inner, b, d_tile]
    ctx_all = singles.tile([P, BL, KT], F32, tag="ctxall", name="ctx_all")

    # ---- main per-example pipeline ----------------------------------------
    # The score/softmax/context phase for example b is EMITTED after example
    # b+1's first u-tile: its 20+4 score matmuls then sit after ut0's in the
    # scheduler's priority order and execute as one clean bundle at the
    # ut0/ut1 boundary (Exp drains while the PE runs the score matmuls), so
    # the weight-stationary j-groups of the big matmul never get split.
    pending = {}

    pending2 = {}

    def emit_context(b):
        attn, rcp, vbf_tile = pending2.pop(b)
        # context[d] = (1/sumexp) * sum_t exp(score_t) * valuesT[d, t]
        ctxp_t = smallp.tile([P, KT], F32, tag="ctxp", name="ctxp_t")
        for dt in range(KT):
            scr = scrp.tile([P, T], BF16, tag="scr", name="scr")
            nc.vector.scalar_tensor_tensor(
                out=scr[:],
                in0=vbf_tile[:, dt, :],
                scalar=1.0,
                in1=attn[:],
                op0=MULT,
                op1=MULT,
                accum_out=ctxp_t[:, dt : dt + 1],
            )
        nc.vector.tensor_scalar_mul(out=ctx_all[:, b, :], in0=ctxp_t[:], scalar1=rcp[:])

    def emit_score_softmax_context(b, score_ps, defer_context=False):
        acc, gs, vbf_tile = pending.pop(b)
        # score[t] = sum_p (accA + g4 + ... + g7)[p, t], broadcast to 128
        # partitions, then softmax over T (no max subtraction;
        # |score| <= sum|V_k| ~ 16). The broadcast reuses ut7's psum banks
        # (write-after-tanh) so the score phase allocates NO extra psum
        # tiles and the bank<->tile mapping stays periodic across examples.
        summands = [acc] + gs
        attn = attnp.tile([P, T], BF16, tag="attn", name="attn")
        sumexp = smallp.tile([P, 1], F32, tag="sumexp", name="sumexp")
        for si, s in enumerate(summands):
            for c in range(NQ):
                nc.tensor.matmul(
                    score_ps[:, c * QT : (c + 1) * QT],
                    lhsT=ones_sb[:],
                    rhs=s[:, c * QT : (c + 1) * QT],
                    start=(si == 0),
                    stop=(si == len(summands) - 1),
                )
        nc.scalar.activation(
            out=attn[:],
            in_=score_ps[:],
            func=AF.Exp,
            accum_out=sumexp[:],
        )
        rcp = smallp.tile([P, 1], F32, tag="rcp", name="rcp")
        nc.vector.reciprocal(out=rcp[:], in_=sumexp[:])

        pending2[b] = (attn, rcp, vbf_tile)
        if not defer_context:
            emit_context(b)

    for b in range(BL):
        prefetch = []
        if b + 1 < BL:
            ld2 = load_vbf if b + 1 < BL - 1 else (lambda _b: load_vnat())
            prefetch = [(load_v8, b + 1), (ld2, b + 1)]

        # score partials: V-weighted tanh tiles. ut 0..3 are chained on DVE
        # into accA; ut 4..7 stay as separate g tiles (pre-multiplied by V at
        # 2x tensor_scalar rate) and the PE's score matmul sums them -- this
        # keeps the post-tanh serial DVE latency off the critical path.
        acc = None
        gs = []
        psv0 = None
        for ut in range(UT):
            if ut in (2, 5) and prefetch:
                fn, arg = prefetch.pop(0)
                fn(arg)
            # v_projT[u, t] for this u-tile: fp8 DoubleRow, 2 k-tiles/pass.
            # One [P, 2048] psum tile spans 4 banks; the c-chunks are
            # independent 512-wide accumulation groups within it, consumed by
            # a single merged tanh activation.
            psv = vpsum.tile([P, T], F32, tag="vp", name="psv")
            for j in range(PAIRS):
                for c in range(NQ):
                    nc.tensor.matmul(
                        psv[:, c * QT : (c + 1) * QT],
                        lhsT=w28_sb[:, 2 * j : 2 * j + 2, ut * P : (ut + 1) * P],
                        rhs=v8_t[b][:, 2 * j : 2 * j + 2, c * QT : (c + 1) * QT],
                        start=(j == 0),
                        stop=(j == PAIRS - 1),
                        perf_mode=DR,
                    )
            th = thp.tile([P, T], BF16, tag="th", name="th")
            nc.scalar.activation(
                out=th[:],
                in_=psv[:],
                func=AF.Tanh,
                bias=qb_sb[:, ut, b : b + 1],
                scale=desc_sb[:, 0:1],
            )
            if ut == 0:
                psv0 = psv
            if ut == 0:
                acc = accp.tile([P, T], BF16, tag="acc", name="acc")
                nc.vector.tensor_scalar(
                    out=acc[:], in0=th[:], scalar1=vk_sb[:, 0:1], scalar2=None,
                    op0=MULT,
                )
            elif ut < NCHAIN:
                nacc = accp.tile([P, T], BF16, tag="acc", name="acc")
                nc.vector.scalar_tensor_tensor(
                    out=nacc[:],
                    in0=th[:],
                    scalar=vk_sb[:, ut : ut + 1],
                    in1=acc[:],
                    op0=MULT,
                    op1=ADD,
                )
                acc = nacc
            else:
                g = gp.tile([P, T], BF16, tag="g", name=f"g{ut}")
                nc.vector.tensor_scalar(
                    out=g[:], in0=th[:], scalar1=vk_sb[:, ut : ut + 1], scalar2=None,
                    op0=MULT,
                )
                gs.append(g)
            if ut == 1 and b > 0:
                emit_score_softmax_context(
                    b - 1, psv0, defer_context=(b == BL - 1)
                )
        while prefetch:
            fn, arg = prefetch.pop(0)
            fn(arg)
        pending[b] = (acc, gs, vbf_t.get(b))
    emit_context(BL - 2)

    # ---- last example: score -> bf16 copy -> scoreT via N=1 matmuls ->
    # exp(scale=1/128) -> PE matvec over natural-layout values ------------
    b = BL - 1
    acc, gs, _ = pending.pop(b)
    summands = [acc] + gs
    final_ps = vpsum.tile([P, T], F32, tag="vp", name="final_ps")
    for si, s in enumerate(summands):
        for c in range(NQ):
            nc.tensor.matmul(
                final_ps[0:1, c * QT : (c + 1) * QT],
                lhsT=ones_sb[:, 0:1],
                rhs=s[:, c * QT : (c + 1) * QT],
                start=(si == 0),
                stop=(si == len(summands) - 1),
            )
    # psum -> sbuf bf16 score copy, chunked so the scoreT matmuls can trail
    score3 = attnp.tile([P, T], BF16, tag="attn", name="score3")
    for c in range(NQ):
        nc.scalar.activation(
            out=score3[0:1, c * QT : (c + 1) * QT],
            in_=final_ps[0:1, c * QT : (c + 1) * QT],
            func=AF.Copy,
        )
    aux = vpsum.tile([P, T], F32, tag="vp", name="aux")
    TCH = T // P  # 16 t-chunks
    # all broadcast rows are identical, so a ones-column reduce gives
    # 128*score per t-lane; the exact 1/128 folds into exp's scale
    for tc in range(TCH):
        nc.tensor.matmul(
            aux[:, tc : tc + 1],
            lhsT=score3[0:1, tc * P : (tc + 1) * P],
            rhs=ones_sb[0:1, 0:1],
            start=True,
            stop=True,
        )
    attnT = smallp.tile([P, TCH], BF16, tag="attnT", name="attnT")
    # exp in halves so the first matvec chunks start while the second half
    # of scoreT is still being extracted
    for h in range(2):
        nc.scalar.activation(
            out=attnT[:, h * TCH // 2 : (h + 1) * TCH // 2],
            in_=aux[:, h * TCH // 2 : (h + 1) * TCH // 2],
            func=AF.Exp,
        )
    # context matvec: ctx[d] = sum_tc sum_t attnT[t, tc] * vnat[t, tc, d]
    vn = vnat_t[0]
    for tc in range(TCH):
        for h in range(2):
            nc.tensor.matmul(
                aux[0:1, (2 + h) * QT : (2 + h) * QT + QT],
                lhsT=attnT[:, tc : tc + 1],
                rhs=vn[:, tc, h * QT : (h + 1) * QT],
                start=(tc == 0),
                stop=(tc == TCH - 1),
            )
    # sum of exp (needed only for the final scale): partition-reduce attnT,
    # then reduce the 16 chunk sums
    nc.tensor.matmul(
        aux[0:1, QT : QT + TCH], lhsT=ones_sb[:, 0:1], rhs=attnT[:],
        start=True, stop=True,
    )
    se3 = smallp.tile([1, 1], F32, tag="se3", name="se3")
    nc.vector.reduce_sum(out=se3[:], in_=aux[0:1, QT : QT + TCH], axis=AX.X)
    rcp3 = smallp.tile([1, 1], F32, tag="rcp3", name="rcp3")
    nc.vector.reciprocal(out=rcp3[:], in_=se3[:])
    ctx3 = smallp.tile([1, D], F32, tag="ctx3", name="ctx3")
    nc.scalar.activation(
        out=ctx3[:], in_=aux[0:1, 2 * QT : 2 * QT + D], func=AF.Copy,
        scale=rcp3[0:1, 0:1],
    )
    nc.sync.dma_start(out=out[b : b + 1], in_=ctx3[:])

    # ---- write out examples 0..BL-2: transpose -> contiguous rows --------
    nb = BL - 1
    pso = vpsum.tile([P, T], F32, tag="vp", name="pso")
    nc.tensor.transpose(
        pso[: nb * KT, :P],
        ctx_all[:, :nb, :].rearrange("p b k -> p (b k)"),
        ident32[:],
    )
    ctxT = smallp.tile([nb * KT, P], F32, tag="ctxT", name="ctxT")
    nc.vector.tensor_copy(out=ctxT[:], in_=pso[: nb * KT, :P])
    nc.sync.dma_start(
        out=out[:nb].rearrange("b (dt p) -> (b dt) p", p=P), in_=ctxT[:]
    )


def _dedupe_ldweights(nc: bass.Bass) -> int:
    """Replace an InstLdweights whose stationary operand is identical to the
    previous InstLdweights on the same engine (with only matmuls in between)
    by a NoOp carrying the same name + sync_info. The PE weight registers
    persist across matmuls, so reloading the same tile is pure overhead
    (~114 ns each) that serializes with the matmul stream.

    Semaphore waits/updates, drains, noops, and register moves do not touch
    the PE weight registers, so tracking survives them; the SBUF region the
    weights were loaded from is only safe to skip re-reading because weight
    tiles here (w28_sb, ones_sb, ident32) are written once and never reused
    for anything else."""
    TRANSPARENT = {
        "InstEventSemaphore",
        "InstDrain",
        "InstNoOp",
        "InstRegisterMove",
    }
    n = 0
    for f in nc.m.functions:
        for blk in f.blocks:
            il = blk.instructions
            last_sig = {}
            out = []
            for inst in il:
                tn = type(inst).__name__
                eng = getattr(inst, "engine", None)
                if tn == "InstLdweights":
                    op = inst.ins[0]
                    sig = (
                        getattr(op, "memref", None),
                        getattr(op, "offset", None),
                        str(getattr(op, "ap", None)),
                        str(getattr(op, "dtype", None)),
                        str(inst.is_transpose),
                        str(inst.perf_mode),
                        str(inst.tile_position),
                    )
                    if last_sig.get(eng) == sig:
                        out.append(
                            mybir.InstNoOp(
                                name=inst.name,
                                engine=inst.engine,
                                ins=[],
                                outs=[],
                                sync_info=inst.sync_info,
                            )
                        )
                        n += 1
                        continue
                    last_sig[eng] = sig
                elif (
                    tn != "InstMatmult" and tn not in TRANSPARENT and eng is not None
                ):
                    # anything else on this engine invalidates tracking
                    last_sig.pop(eng, None)
                out.append(inst)
            il[:] = out
    return n


def _split_multi_waits(nc: bass.Bass) -> int:
    """The walrus build here accepts only ONE semaphore wait per instruction;
    hoist extra waits onto single-wait NoOps preceding the instruction (same
    engine, in-order, so semantics are preserved)."""
    n_split = 0
    for f in nc.m.functions:
        for b in f.blocks:
            il = b.instructions
            out, changed = [], False
            for inst in il:
                si = inst.sync_info
                waits = list(si.on_wait) if (si and si.on_wait) else []
                if len(waits) > 1:
                    changed = True
                    n_split += 1
                    for j, w in enumerate(waits[:-1]):
                        out.append(
                            mybir.InstNoOp(
                                name=f"{inst.name}.sw{j}",
                                engine=inst.engine,
                                ins=[],
                                outs=[],
                                sync_info=mybir.SyncInfo(on_wait=[w], on_update=[]),
                            )
                        )
                    inst.sync_info = mybir.SyncInfo(
                        on_wait=[waits[-1]], on_update=list(si.on_update or [])
                    )
                out.append(inst)
            if changed:
                il[:] = out
    return n_split


def build_program(split_waits: bool = True) -> bass.Bass:
    nc = bass.Bass("TRN2", target_bir_lowering=False, debug=False, num_devices=NCORES)
    qb_h = nc.dram_tensor("qb", [P, UT, BL], F32, kind="ExternalInput")
    v8_h = nc.dram_tensor("values8", [BL, P, KT, T], F8, kind="ExternalInput")
    vbf_h = nc.dram_tensor("valuesbf", [BL, P, KT, T], BF16, kind="ExternalInput")
    vnat_h = nc.dram_tensor("valuesnat", [P, T // P, D], BF16, kind="ExternalInput")
    w28_h = nc.dram_tensor("W2_8", [P, KT, U], F8, kind="ExternalInput")
    vk_h = nc.dram_tensor("Vk", [P, UT], F32, kind="ExternalInput")
    desc_h = nc.dram_tensor("descale", [P, 1], F32, kind="ExternalInput")
    out_h = nc.dram_tensor("context", [BL, D], F32, kind="ExternalOutput")
    with tile.TileContext(nc) as tc:
        with ExitStack() as ctx:
            _emit(
                ctx, tc,
                qb_h.ap(), v8_h.ap(), vbf_h.ap(), vnat_h.ap(),
                w28_h.ap(), vk_h.ap(), desc_h.ap(),
                out_h.ap(),
            )
    if split_waits:
        _dedupe_ldweights(nc)
        _split_multi_waits(nc)
    return nc


_PROGRAM = None


def _get_program() -> bass.Bass:
    global _PROGRAM
    if _PROGRAM is None:
        _PROGRAM = build_program()
    return _PROGRAM


def _pow2_scale(absmax: float, target: float = 224.0) -> float:
    return float(2.0 ** np.floor(np.log2(target / absmax)))


def make_in_maps(inputs: dict) -> list[dict]:
    import ml_dtypes

    F8NP = ml_dtypes.float8_e4m3
    BFNP = ml_dtypes.bfloat16

    query = np.asarray(inputs["query"], dtype=np.float32)
    values = np.asarray(inputs["values"], dtype=np.float32)
    W1 = np.asarray(inputs["W1_k"], dtype=np.float32)
    W1b = np.asarray(inputs["W1_b"], dtype=np.float32)
    W2 = np.asarray(inputs["W2_k"], dtype=np.float32)
    W2b = np.asarray(inputs["W2_b"], dtype=np.float32)
    Vk = np.asarray(inputs["V_k"], dtype=np.float32)

    sv = _pow2_scale(float(np.abs(values).max()))
    sw = _pow2_scale(float(np.abs(W2).max()))

    # values, transposed to [b, p, kt, t]
    vT = np.ascontiguousarray(values.transpose(0, 2, 1))  # [B, D, T]
    vT = vT.reshape(B, KT, P, T).swapaxes(1, 2)  # [B, P, KT, T] view
    vbf = vT.astype(BFNP)
    v8 = (vT * np.float32(sv)).astype(F8NP)

    # W2 * sw, [p, kt, u] fp8
    w28 = (W2 * np.float32(sw)).reshape(KT, P, U).swapaxes(0, 1).astype(F8NP)
    w28 = np.ascontiguousarray(w28)

    # qb = query @ W1 + W1_b + W2_b, folded host-side (fp32), [p, ut, b]
    qb_full = query @ W1 + (W1b + W2b)[None, :]  # [B, U]

    vk = np.ascontiguousarray(Vk[:, 0].reshape(UT, P).T, dtype=np.float32)
    desc = np.full((P, 1), 1.0 / (sv * sw), dtype=np.float32)

    TCH = T // P
    in_maps = []
    for c in range(NCORES):
        sl = slice(c * BL, (c + 1) * BL)
        vnat_c = np.ascontiguousarray(
            values[c * BL + BL - 1].reshape(TCH, P, D).swapaxes(0, 1).astype(BFNP)
        )
        qb_c = np.ascontiguousarray(
            qb_full[sl].T.reshape(UT, P, BL).swapaxes(0, 1), dtype=np.float32
        )
        in_maps.append(
            {
                "qb": qb_c,
                "valuesnat": vnat_c,
                "values8": np.ascontiguousarray(v8[sl]),
                "valuesbf": np.ascontiguousarray(vbf[sl]),
                "W2_8": w28,
                "Vk": vk,
                "descale": desc,
            }
        )
    return in_maps


def kernel(**inputs) -> np.ndarray:
    nc = _get_program()
    res = run_bass_kernel_spmd(nc, make_in_maps(inputs), list(range(NCORES))).results
    return np.concatenate([res[c]["context"] for c in range(NCORES)], axis=0)


if __name__ == "__main__":
    rng = np.random.default_rng(0)
    inputs = {
        "query": rng.standard_normal((B, D), dtype=np.float32),
        "values": rng.standard_normal((B, T, D), dtype=np.float32),
        "W1_k": (rng.standard_normal((D, U)) * 0.02).astype(np.float32),
        "W1_b": np.zeros(U, np.float32),
        "W2_k": (rng.standard_normal((D, U)) * 0.02).astype(np.float32),
        "W2_b": np.zeros(U, np.float32),
        "V_k": (rng.standard_normal((U, 1)) * 0.02).astype(np.float32),
        "V_b": np.zeros(1, np.float32),
    }
    out = kernel(**inputs)
    print(out.shape, out.dtype)
